# revision 1
# baseline (speedup 1.0000x reference)
"""Trainium2 Bass kernel for DAS (delay-and-sum) ultrasound beamforming.

Math: the per-(t,e,z) delay/phase depend on (t,e) only through
vx = gx[t]-ex[e], i.e. on delta = t-e (Toeplitz geometry). Per-delta tables
(gather index i0, fused interp/rotation/apod weights) are computed on host
from the small geometry inputs; the 512MB of sample data is processed on
8 NeuronCores:

  per (core, slot) = one delta diagonal: DMA the diagonal rows (t, t-delta)
  of interleaved I/Q data -> GPSIMD ap_gather at i0 and i0+1 (indices shared
  across partitions = transmits) -> PE transpose to [z, t] -> DVE/ACT
  multiply by per-delta weight columns (free-axis broadcast) and accumulate.
  Host sums the 8 per-core partial [z,t] accumulators.

The apodization mask is validated exactly per (t,e,z) on host; any mismatch
vs the delta-representative mask is fixed with sparse host corrections
(zero for the reference geometry).

SPMD uniformity: 255 deltas + 1 dummy = 256 (core,slot) instances arranged
in 32 slots x 8 cores, grouped by |delta| so every core's slot k has the
same compiled partition extent/offset.
"""
import os
import sys

for _p in ('/opt/trn_rl_repo', '/root/.axon_site/_ro/trn_rl_repo'):
    if os.path.isdir(_p) and _p not in sys.path:
        sys.path.append(_p)

import numpy as np

T, E, S, Z = 128, 128, 4096, 2048
PI = 3.14159265359
MIN_WIDTH = 0.001
N_CORES = 8
N_SLOTS = 32
NBLK = 16          # z blocks of 128
DUMMY = 999


def _f32(x):
    return np.asarray(x, dtype=np.float32)


# ---------------------------------------------------------------- host math
def build_slot_assignment():
    pos = sorted(range(0, 128), key=lambda d: -d)
    neg = [DUMMY] + sorted(range(-127, 0), key=lambda d: d)
    slots = []
    for k in range(16):
        group = pos[8 * k: 8 * k + 8]
        toff = min(group)
        slots.append(dict(toff=toff, ext=128 - toff, deltas=group))
    for k in range(16):
        group = neg[8 * k: 8 * k + 8]
        real = [d for d in group if d != DUMMY]
        ext = 128 - min(abs(d) for d in real)
        slots.append(dict(toff=0, ext=ext, deltas=group))
    return slots


def compute_tables(grid, tx_ori, ele_pos, time_zero, fs, c, fdemod, rxfnum):
    grid = _f32(grid); tx_ori = _f32(tx_ori); ele_pos = _f32(ele_pos)
    time_zero = _f32(time_zero)
    gx = grid[:, 0, 0]
    zax = grid[0, :, 2]
    ex = ele_pos[:, 0]

    vx_te = (gx[:, None] - ex[None, :]).astype(np.float32)
    vz = zax.astype(np.float32)
    with np.errstate(divide='ignore', invalid='ignore'):
        ratio = np.abs(vz[None, None, :] / vx_te[:, :, None])
    m = ratio > np.float32(rxfnum)
    m |= (np.abs(vx_te) <= np.float32(MIN_WIDTH))[:, :, None]
    m |= ((vx_te >= np.float32(MIN_WIDTH)) & (gx[:, None] <= ex[0]))[:, :, None]
    m |= ((vx_te <= np.float32(-MIN_WIDTH)) & (gx[:, None] >= ex[-1]))[:, :, None]
    mask_exact = m

    d3 = grid - tx_ori[:, None, :]
    txdel = np.sqrt((d3 * d3).sum(-1, dtype=np.float32)).astype(np.float32)

    nd = 255
    i0_tab = np.zeros((nd, Z), np.int32)
    frac_tab = np.zeros((nd, Z), np.float32)
    ct_tab = np.zeros((nd, Z), np.float32)
    st_tab = np.zeros((nd, Z), np.float32)
    v0_tab = np.zeros((nd, Z), np.float32)
    v1_tab = np.zeros((nd, Z), np.float32)
    mask_tab = np.zeros((nd, Z), bool)
    for delta in range(-127, 128):
        t_rep = max(0, delta); e_rep = t_rep - delta
        vx = vx_te[t_rep, e_rep]
        rx = np.sqrt(vx * vx + vz * vz).astype(np.float32)
        delays = ((txdel[t_rep] + rx) / np.float32(c)
                  + time_zero[t_rep]) * np.float32(fs)
        i0f = np.floor(delays)
        frac = (delays - i0f).astype(np.float32)
        i0 = i0f.astype(np.int32)
        tshift = delays / np.float32(fs) - zax * np.float32(2.0) / np.float32(c)
        theta = (np.float32(2.0 * PI * fdemod) * tshift).astype(np.float32)
        j = delta + 127
        i0_tab[j] = i0
        frac_tab[j] = frac
        ct_tab[j] = np.cos(theta, dtype=np.float32)
        st_tab[j] = np.sin(theta, dtype=np.float32)
        v0_tab[j] = (i0 >= 0) & (i0 < S)
        v1_tab[j] = (i0 + 1 >= 0) & (i0 + 1 < S)
        mask_tab[j] = mask_exact[t_rep, e_rep]
    return dict(i0=i0_tab, frac=frac_tab, ct=ct_tab, st=st_tab,
                v0=v0_tab, v1=v1_tab, mask_tab=mask_tab,
                mask_exact=mask_exact)


def build_weight_tables(tabs):
    """[255, 6, Z]: (wa, wb, -wc, -wd, wc, wd);
    accI += wa*I0 + wb*I1 - wc*Q0 - wd*Q1
    accQ += wc*I0 + wd*I1 + wa*Q0 + wb*Q1"""
    apod = tabs['mask_tab'].astype(np.float32)
    omf = np.float32(1.0) - tabs['frac']
    wa = apod * tabs['ct'] * omf * tabs['v0']
    wb = apod * tabs['ct'] * tabs['frac'] * tabs['v1']
    wc = apod * tabs['st'] * omf * tabs['v0']
    wd = apod * tabs['st'] * tabs['frac'] * tabs['v1']
    return np.stack([wa, wb, -wc, -wd, wc, wd], axis=1).astype(np.float32)


def corrections(idata, qdata, tabs):
    corrI = np.zeros((T, Z), np.float32)
    corrQ = np.zeros((T, Z), np.float32)
    i0c = np.clip(tabs['i0'], 0, S - 1)
    i1c = np.clip(tabs['i0'] + 1, 0, S - 1)
    for delta in range(-127, 128):
        j = delta + 127
        ts = np.arange(max(0, delta), min(T - 1, T - 1 + delta) + 1)
        es = ts - delta
        dm = (tabs['mask_exact'][ts, es, :].astype(np.int8)
              - tabs['mask_tab'][j][None, :].astype(np.int8))
        nz = np.argwhere(dm != 0)
        if nz.size == 0:
            continue
        ti, zi = nz[:, 0], nz[:, 1]
        tt, ee = ts[ti], es[ti]
        sgn = dm[ti, zi].astype(np.float32)
        f = tabs['frac'][j][zi]; ct = tabs['ct'][j][zi]; st = tabs['st'][j][zi]
        v0 = tabs['v0'][j][zi]; v1 = tabs['v1'][j][zi]
        I0 = idata[tt, ee, i0c[j][zi]] * v0; I1 = idata[tt, ee, i1c[j][zi]] * v1
        Q0 = qdata[tt, ee, i0c[j][zi]] * v0; Q1 = qdata[tt, ee, i1c[j][zi]] * v1
        fi = (1 - f) * I0 + f * I1
        fq = (1 - f) * Q0 + f * Q1
        np.add.at(corrI, (tt, zi), sgn * (ct * fi - st * fq))
        np.add.at(corrQ, (tt, zi), sgn * (ct * fq + st * fi))
    return corrI, corrQ


# ------------------------------------------------------------- bass program
_CACHE = {}


def _build_program(slots):
    import concourse.bacc as bacc
    import concourse.mybir as mybir
    from concourse.tile import TileContext
    from concourse.masks import make_identity

    DT = mybir.dt.float32
    r_tot = sum(sl['ext'] for sl in slots)
    nc = bacc.Bacc("TRN2", target_bir_lowering=False, debug=False,
                   num_devices=N_CORES)
    rows_d = nc.dram_tensor("rows", [r_tot, S * 2], DT, kind="ExternalInput").ap()
    idx_d = nc.dram_tensor("idx", [N_SLOTS, 128, 256], mybir.dt.int16,
                           kind="ExternalInput").ap()
    wts_d = nc.dram_tensor("wts", [N_SLOTS, 128, 96], DT,
                           kind="ExternalInput").ap()
    accI_d = nc.dram_tensor("accI", [128, Z], DT, kind="ExternalOutput").ap()
    accQ_d = nc.dram_tensor("accQ", [128, Z], DT, kind="ExternalOutput").ap()

    with TileContext(nc) as tc:
        with tc.tile_pool(name="data", bufs=2) as dpool, \
             tc.tile_pool(name="gout", bufs=2) as gpool, \
             tc.tile_pool(name="small", bufs=2) as spool, \
             tc.tile_pool(name="tmp", bufs=3) as tpool, \
             tc.tile_pool(name="accp", bufs=1) as apool, \
             tc.tile_pool(name="psum", bufs=2, space="PSUM") as ppool:
            ident = apool.tile([128, 128], DT, tag="ident")
            make_identity(nc, ident[:])
            accI = apool.tile([128, NBLK, 128], DT, tag="accI")
            accQ = apool.tile([128, NBLK, 128], DT, tag="accQ")
            nc.vector.memset(accI[:], 0.0)
            nc.vector.memset(accQ[:], 0.0)

            row_off = 0
            for k, sl in enumerate(slots):
                ext, toff = sl['ext'], sl['toff']
                data_t = dpool.tile([128, S, 2], DT, tag="data")
                nc.sync.dma_start(out=data_t[0:ext],
                                  in_=rows_d[row_off:row_off + ext])
                idx_t = spool.tile([128, 256], mybir.dt.int16, tag="idx")
                nc.sync.dma_start(out=idx_t[:], in_=idx_d[k])
                w_t = spool.tile([128, 96], DT, tag="wts")
                nc.sync.dma_start(out=w_t[:], in_=wts_d[k])

                gout0 = gpool.tile([128, Z, 2], DT, tag="g0")
                gout1 = gpool.tile([128, Z, 2], DT, tag="g1")
                nc.gpsimd.ap_gather(gout0[:], data_t[:], idx_t[:, 0:128],
                                    channels=128, num_elems=S, d=2,
                                    num_idxs=Z)
                nc.gpsimd.ap_gather(gout1[:], data_t[:], idx_t[:, 128:256],
                                    channels=128, num_elems=S, d=2,
                                    num_idxs=Z)

                # (source tile, IQ channel, accI table idx, accQ table idx)
                for (src, ch, tabI, tabQ) in ((gout0, 0, 0, 4),
                                              (gout1, 0, 1, 5),
                                              (gout0, 1, 2, 0),
                                              (gout1, 1, 3, 1)):
                    big = ppool.tile([128, NBLK, 128], DT, space="PSUM",
                                     tag="big")
                    for blk in range(NBLK):
                        nc.tensor.transpose(
                            out=big[:, blk, :],
                            in_=src[:, blk * 128:(blk + 1) * 128, ch],
                            identity=ident[:])
                    for (acc, tab) in ((accI, tabI), (accQ, tabQ)):
                        w_ap = w_t[:, tab * 16:(tab + 1) * 16] \
                            .broadcast_to([128, NBLK, ext])
                        tmp = tpool.tile([128, NBLK, 128], DT, tag="tmp")
                        nc.any.tensor_tensor(
                            out=tmp[:, :, 0:ext], in0=big[:, :, 0:ext],
                            in1=w_ap, op=mybir.AluOpType.mult)
                        nc.any.tensor_tensor(
                            out=acc[:, :, toff:toff + ext],
                            in0=acc[:, :, toff:toff + ext],
                            in1=tmp[:, :, 0:ext], op=mybir.AluOpType.add)
                row_off += ext

            nc.sync.dma_start(out=accI_d[:], in_=accI[:])
            nc.sync.dma_start(out=accQ_d[:], in_=accQ[:])
    nc.compile()
    return nc


def _get_program_and_slots():
    if 'prog' not in _CACHE:
        slots = build_slot_assignment()
        _CACHE['slots'] = slots
        _CACHE['prog'] = _build_program(slots)
    return _CACHE['prog'], _CACHE['slots']


def _pack_inputs(idata, qdata, tabs, wtabs, slots):
    """Per-core input dicts."""
    data_iq = np.empty((T * E, S * 2), np.float32)
    data_iq[:, 0::2] = idata.reshape(T * E, S)
    data_iq[:, 1::2] = qdata.reshape(T * E, S)

    i0c = np.clip(tabs['i0'], 0, S - 1).astype(np.int16)
    i1c = np.clip(tabs['i0'] + 1, 0, S - 1).astype(np.int16)
    # wrapped idx layout: wrapped[p, s] = idx[s*16 + p%16]
    pp = (np.arange(128)[:, None] % 16)
    ss = np.arange(128)[None, :] * 16
    wrap_sel = (ss + pp)                      # [128,128]

    r_tot = sum(sl['ext'] for sl in slots)
    in_maps = []
    for c in range(N_CORES):
        rows = np.zeros((r_tot, S * 2), np.float32)
        idx = np.zeros((N_SLOTS, 128, 256), np.int16)
        wts = np.zeros((N_SLOTS, 128, 96), np.float32)
        row_off = 0
        for k, sl in enumerate(slots):
            ext, toff = sl['ext'], sl['toff']
            delta = sl['deltas'][c]
            if delta != DUMMY:
                j = delta + 127
                if delta >= 0:
                    ts = np.arange(delta, T)
                else:
                    ts = np.arange(0, T + delta)
                ps = ts - toff
                rows[row_off + ps] = data_iq[ts * E + (ts - delta)]
                idx[k, :, 0:128] = i0c[j][wrap_sel]
                idx[k, :, 128:256] = i1c[j][wrap_sel]
                # wts[k, p, tab*16+blk] = wtabs[j, tab, blk*128+p]
                wts[k] = wtabs[j].reshape(6, NBLK, 128) \
                    .transpose(2, 0, 1).reshape(128, 96)
            row_off += ext
        in_maps.append({"rows": rows, "idx": idx, "wts": wts})
    return in_maps


def kernel(idata, qdata, grid, tx_ori, ele_pos, time_zero,
           fs, c, fdemod, rxfnum):
    from concourse.bass_utils import run_bass_kernel_spmd

    idata = _f32(idata); qdata = _f32(qdata)
    tabs = compute_tables(grid, tx_ori, ele_pos, time_zero,
                          fs, c, fdemod, rxfnum)
    wtabs = build_weight_tables(tabs)
    nc, slots = _get_program_and_slots()
    in_maps = _pack_inputs(idata, qdata, tabs, wtabs, slots)
    res = run_bass_kernel_spmd(nc, in_maps, list(range(N_CORES)))
    _CACHE['last_results'] = res

    idas = np.zeros((T, Z), np.float32)
    qdas = np.zeros((T, Z), np.float32)
    for cidx in range(N_CORES):
        aI = res.results[cidx]["accI"].reshape(128, NBLK, 128)
        aQ = res.results[cidx]["accQ"].reshape(128, NBLK, 128)
        idas += aI.transpose(1, 0, 2).reshape(Z, T).T
        qdas += aQ.transpose(1, 0, 2).reshape(Z, T).T
    cI, cQ = corrections(idata, qdata, tabs)
    idas += cI
    qdas += cQ
    return (idas, qdas)



# revision 4
# speedup vs baseline: 1.4245x; 1.4245x over previous
"""Trainium2 Bass kernel for DAS (delay-and-sum) ultrasound beamforming.

Wire-optimized rewrite of the diagonal (Toeplitz) scheme: the per-(t,e,z)
delay/phase depend on (t,e) only through delta = t-e, so per-delta tables
drive a shared gather on all 128 rows of a diagonal.

Key wall-clock levers vs the f32 baseline (the axon tunnel moves ~60MB/s,
so bytes-on-the-wire dominate):
  1. delta pruning: the dynamic-aperture apod mask is identically zero for
     |delta| >= 100 (needs z > rxfnum*|vx|, z_max = 60mm) -> only 199 of 255
     diagonals are shipped/computed.
  2. z-windowing: per diagonal only z >= 0.6*|delta|mm contribute; gathers,
     weights and accumulation are restricted to the active 128-z blocks.
  3. sample-windowing: per diagonal only samples i0(z_lo)..i0(z_hi)+1 are
     ever gathered (a ~250..1450-wide window of the 4096) -> ship only the
     window, with indices rebased.
  4. int8 quantization with per-row scales: I in the low byte (biased
     uint8), Q in the high byte (signed int8) of one int16 word; unpacked
     on-device with and/sub/mult DVE ops. Gathers fetch int16 PAIRS of
     consecutive samples (d=2) and host-folded parity weights select the
     (i0, i0+1) interpolation pair from the 3 distinct lanes.
  5. fp16 weight tables and fp16 outputs.

Total wire: ~35MB vs 590MB for the f32 baseline.
"""
import os
import sys

for _p in ('/opt/trn_rl_repo', '/root/.axon_site/_ro/trn_rl_repo'):
    if os.path.isdir(_p) and _p not in sys.path:
        sys.path.append(_p)

import numpy as np

T, E, S, Z = 128, 128, 4096, 2048
PI = 3.14159265359
MIN_WIDTH = 0.001
N_CORES = 8
NBLK = 16
CB = 8            # z-blocks per processing chunk
DUMMY = 999


def _f32(x):
    return np.asarray(x, dtype=np.float32)


# ---------------------------------------------------------------- host math
def compute_tables(grid, tx_ori, ele_pos, time_zero, fs, c, fdemod, rxfnum):
    grid = _f32(grid); tx_ori = _f32(tx_ori); ele_pos = _f32(ele_pos)
    time_zero = _f32(time_zero)
    gx = grid[:, 0, 0]
    zax = grid[0, :, 2]
    ex = ele_pos[:, 0]

    vx_te = (gx[:, None] - ex[None, :]).astype(np.float32)
    vz = zax.astype(np.float32)
    with np.errstate(divide='ignore', invalid='ignore'):
        ratio = np.abs(vz[None, None, :] / vx_te[:, :, None])
    m = ratio > np.float32(rxfnum)
    m |= (np.abs(vx_te) <= np.float32(MIN_WIDTH))[:, :, None]
    m |= ((vx_te >= np.float32(MIN_WIDTH)) & (gx[:, None] <= ex[0]))[:, :, None]
    m |= ((vx_te <= np.float32(-MIN_WIDTH)) & (gx[:, None] >= ex[-1]))[:, :, None]
    mask_exact = m

    d3 = grid - tx_ori[:, None, :]
    txdel = np.sqrt((d3 * d3).sum(-1, dtype=np.float32)).astype(np.float32)

    nd = 255
    i0_tab = np.zeros((nd, Z), np.int32)
    frac_tab = np.zeros((nd, Z), np.float32)
    ct_tab = np.zeros((nd, Z), np.float32)
    st_tab = np.zeros((nd, Z), np.float32)
    v0_tab = np.zeros((nd, Z), np.float32)
    v1_tab = np.zeros((nd, Z), np.float32)
    mask_tab = np.zeros((nd, Z), bool)
    for delta in range(-127, 128):
        t_rep = max(0, delta); e_rep = t_rep - delta
        vx = vx_te[t_rep, e_rep]
        rx = np.sqrt(vx * vx + vz * vz).astype(np.float32)
        delays = ((txdel[t_rep] + rx) / np.float32(c)
                  + time_zero[t_rep]) * np.float32(fs)
        i0f = np.floor(delays)
        frac = (delays - i0f).astype(np.float32)
        i0 = i0f.astype(np.int32)
        tshift = delays / np.float32(fs) - zax * np.float32(2.0) / np.float32(c)
        theta = (np.float32(2.0 * PI * fdemod) * tshift).astype(np.float32)
        j = delta + 127
        i0_tab[j] = i0
        frac_tab[j] = frac
        ct_tab[j] = np.cos(theta, dtype=np.float32)
        st_tab[j] = np.sin(theta, dtype=np.float32)
        v0_tab[j] = (i0 >= 0) & (i0 < S)
        v1_tab[j] = (i0 + 1 >= 0) & (i0 + 1 < S)
        mask_tab[j] = mask_exact[t_rep, e_rep]
    return dict(i0=i0_tab, frac=frac_tab, ct=ct_tab, st=st_tab,
                v0=v0_tab, v1=v1_tab, mask_tab=mask_tab,
                mask_exact=mask_exact)


def build_plan(tabs):
    """Slot assignment: active deltas grouped 8 per slot by family (pos/neg)
    and descending |delta| (similar window widths group together)."""
    i0 = tabs['i0']; mask = tabs['mask_tab']
    act = {}
    for delta in range(-127, 128):
        j = delta + 127
        zs = np.where(mask[j])[0]
        if len(zs) == 0:
            continue
        assert len(zs) == zs.max() - zs.min() + 1, "active z not contiguous"
        blk0 = int(zs.min()) // 128
        base = int(i0[j, zs.min()]) & ~1  # even
        base = max(base, 0)
        last = int(i0[j, Z - 1])
        assert last + 1 < S, "gather window exceeds data"
        W = last + 2 - base
        act[delta] = dict(blk0=blk0, base=base, W=W)

    pos = sorted([d for d in act if d >= 1], key=lambda d: -d)
    neg = sorted([d for d in act if d <= 0], key=lambda d: abs(d), reverse=True)

    def mkslots(fam, is_pos):
        ns = (len(fam) + 7) // 8
        pad = ns * 8 - len(fam)
        fam = fam[:8 - pad] + [DUMMY] * pad + fam[8 - pad:] if pad else fam
        slots = []
        for k in range(ns):
            grp = fam[8 * k: 8 * k + 8]
            real = [d for d in grp if d != DUMMY]
            ext = 128 - min(abs(d) for d in real)
            toff = min(real) if is_pos else 0
            blk0 = min(act[d]['blk0'] for d in real)
            Wp = max(-(-(act[d]['W']) // 2) for d in real) + 1
            nb = NBLK - blk0
            slots.append(dict(deltas=grp, ext=ext, toff=toff, blk0=blk0,
                              Wp=Wp, nb=nb))
        return slots

    slots = mkslots(pos, True) + mkslots(neg, False)
    return dict(slots=slots, act=act)


def build_weight_streams(tabs, plan):
    """Per (slot, core): 6 fp16 weight streams [6, nb*128] with the
    pair-parity fold:
      k_a = (i0-base)>>1 gathers lanes (a0,a1); k_b = (i0+1-base)>>1 lane b0
      p = (i0-base)&1:  I0 = p? a1 : a0 ; I1 = p? b0 : a1
      W1 = wa*(1-p); W2 = wa*p + wb*(1-p); W3 = wb*p  (same V* with wc,wd)
      accI += W1*Ia0 + W2*Ia1 + W3*Ib0 - V1*Qa0 - V2*Qa1 - V3*Qb0
      accQ += V1*Ia0 + V2*Ia1 + V3*Ib0 + W1*Qa0 + W2*Qa1 + W3*Qb0
    Also the rebased wrapped gather index tables."""
    apod = tabs['mask_tab'].astype(np.float32)
    omf = np.float32(1.0) - tabs['frac']
    wa_t = apod * tabs['ct'] * omf * tabs['v0']
    wb_t = apod * tabs['ct'] * tabs['frac'] * tabs['v1']
    wc_t = apod * tabs['st'] * omf * tabs['v0']
    wd_t = apod * tabs['st'] * tabs['frac'] * tabs['v1']

    out = {}
    for delta, a in plan['act'].items():
        j = delta + 127
        blk0, base = a['blk0'], a['base']
        # NOTE: streams/indices are built on the delta's own active window;
        # slots may extend lower (slot blk0 <= delta blk0) - handled at pack
        # time by zero weights and clipped indices.
        out[delta] = dict(wa=wa_t[j], wb=wb_t[j], wc=wc_t[j], wd=wd_t[j])
    return out


def pack_inputs(idata, qdata, tabs, plan):
    """Quantize, window, pack the per-core input dicts."""
    slots = plan['slots']; act = plan['act']
    wtabs = build_weight_streams(tabs, plan)
    i0_tab = tabs['i0']

    in_maps = [dict() for _ in range(N_CORES)]
    for k, sl in enumerate(slots):
        ext, toff, blk0, Wp, nb = (sl['ext'], sl['toff'], sl['blk0'],
                                   sl['Wp'], sl['nb'])
        nidx = nb * 128
        c16 = nidx // 16
        zsel = np.arange(blk0 * 128, Z)
        for cidx in range(N_CORES):
            delta = sl['deltas'][cidx]
            rows = np.zeros((ext, Wp, 2), np.int16)
            idxw = np.zeros((2, 16, c16), np.int16)
            wts = np.zeros((128, 6, nb), np.float16)
            scl = np.zeros((128, 2), np.float32)
            if delta != DUMMY:
                j = delta + 127
                a = act[delta]
                base = a['base']
                if delta >= 0:
                    ts = np.arange(delta, T)
                else:
                    ts = np.arange(0, T + delta)
                es = ts - delta
                ps = ts - toff
                hi = min(base + 2 * Wp, S)
                Iw = idata[ts, es, base:hi]
                Qw = qdata[ts, es, base:hi]
                sI = np.abs(Iw).max(axis=1)
                sQ = np.abs(Qw).max(axis=1)
                sI[sI == 0] = 1.0; sQ[sQ == 0] = 1.0
                qI = np.rint(Iw * (127.0 / sI[:, None])).astype(np.int32)
                qQ = np.rint(Qw * (127.0 / sQ[:, None])).astype(np.int32)
                np.clip(qI, -127, 127, out=qI)
                np.clip(qQ, -127, 127, out=qQ)
                # low byte: I biased to [1,255]; high byte: Q signed
                packed = ((qQ << 8) | (qI + 128)).astype(np.int16)
                rows[ps, :packed.shape[1] // 2, :] = \
                    packed[:, :(packed.shape[1] // 2) * 2].reshape(
                        len(ts), -1, 2)
                scl[ps, 0] = sI / 127.0
                scl[ps, 1] = sQ / (127.0 * 256.0)

                i0 = i0_tab[j][zsel]
                ka = np.clip((i0 - base) >> 1, -1, Wp - 1)
                kb = np.clip((i0 + 1 - base) >> 1, -1, Wp - 1)
                # negative -> ap_gather clamps to elem 0, weight is 0 there
                idxw[0] = ka.astype(np.int16).reshape(c16, 16).T
                idxw[1] = kb.astype(np.int16).reshape(c16, 16).T

                par = ((i0 - base) & 1).astype(np.float32)
                wa = wtabs[delta]['wa'][zsel]; wb = wtabs[delta]['wb'][zsel]
                wc = wtabs[delta]['wc'][zsel]; wd = wtabs[delta]['wd'][zsel]
                om = 1.0 - par
                streams = np.stack([wa * om, wa * par + wb * om, wb * par,
                                    wc * om, wc * par + wd * om, wd * par])
                # below the delta's own active window weights are 0 already
                # wts[p, s, bl] = streams[s, bl*128 + p]
                wts[:] = streams.reshape(6, nb, 128).transpose(2, 0, 1) \
                    .astype(np.float16)
            m = in_maps[cidx]
            m[f"rows{k}"] = rows
            m[f"idx{k}"] = idxw
            m[f"wts{k}"] = wts
            m[f"scl{k}"] = scl
    return in_maps


def corrections(idata, qdata, tabs):
    """Sparse host fix-ups where the per-delta representative mask deviates
    from the exact per-(t,e) mask (zero for the reference geometry)."""
    corrI = np.zeros((T, Z), np.float32)
    corrQ = np.zeros((T, Z), np.float32)
    i0c = np.clip(tabs['i0'], 0, S - 1)
    i1c = np.clip(tabs['i0'] + 1, 0, S - 1)
    for delta in range(-127, 128):
        j = delta + 127
        ts = np.arange(max(0, delta), min(T - 1, T - 1 + delta) + 1)
        es = ts - delta
        dm = (tabs['mask_exact'][ts, es, :].astype(np.int8)
              - tabs['mask_tab'][j][None, :].astype(np.int8))
        nz = np.argwhere(dm != 0)
        if nz.size == 0:
            continue
        ti, zi = nz[:, 0], nz[:, 1]
        tt, ee = ts[ti], es[ti]
        sgn = dm[ti, zi].astype(np.float32)
        f = tabs['frac'][j][zi]; ct = tabs['ct'][j][zi]; st = tabs['st'][j][zi]
        v0 = tabs['v0'][j][zi]; v1 = tabs['v1'][j][zi]
        I0 = idata[tt, ee, i0c[j][zi]] * v0; I1 = idata[tt, ee, i1c[j][zi]] * v1
        Q0 = qdata[tt, ee, i0c[j][zi]] * v0; Q1 = qdata[tt, ee, i1c[j][zi]] * v1
        fi = (1 - f) * I0 + f * I1
        fq = (1 - f) * Q0 + f * Q1
        np.add.at(corrI, (tt, zi), sgn * (ct * fi - st * fq))
        np.add.at(corrQ, (tt, zi), sgn * (ct * fq + st * fi))
    return corrI, corrQ


# ------------------------------------------------------------- bass program
_CACHE = {}


def _make_runner(nc, donate_outputs=False):
    """Like bass2jax.run_bass_via_pjrt, but the traced/jitted executable is
    built once and reused across calls. Output donation is skipped by
    default: this kernel DMAs every element of its outputs, so uninit
    custom-call result buffers are fine and the 8MB zero-buffer upload per
    call is saved."""
    import jax
    from jax.sharding import Mesh, PartitionSpec
    from jax.experimental.shard_map import shard_map
    from concourse import bass2jax
    import concourse.mybir as mybir

    bass2jax.install_neuronx_cc_hook()
    partition_name = (nc.partition_id_tensor.name
                      if nc.partition_id_tensor else None)
    in_names, out_names, out_avals, zero_outs = [], [], [], []
    for alloc in nc.m.functions[0].allocations:
        if not isinstance(alloc, mybir.MemoryLocationSet):
            continue
        name = alloc.memorylocations[0].name
        if alloc.kind == "ExternalInput":
            if name != partition_name:
                in_names.append(name)
        elif alloc.kind == "ExternalOutput":
            out_names.append(name)
            shape = tuple(alloc.tensor_shape)
            dtype = mybir.dt.np(alloc.dtype)
            out_avals.append(jax.core.ShapedArray(shape, dtype))
            zero_outs.append(np.zeros(shape, dtype))
    n_params = len(in_names)
    n_outs = len(out_avals)
    bind_names = list(in_names)
    if donate_outputs:
        bind_names.extend(out_names)
    if partition_name is not None:
        bind_names.append(partition_name)

    def _body(*args):
        operands = list(args)
        if partition_name is not None:
            operands.append(bass2jax.partition_id_tensor())
        outs = bass2jax._bass_exec_p.bind(
            *operands,
            out_avals=tuple(out_avals),
            in_names=tuple(bind_names),
            out_names=tuple(out_names),
            lowering_input_output_aliases=(),
            sim_require_finite=True,
            sim_require_nnan=True,
            nc=nc,
        )
        return tuple(outs)

    devices = jax.devices()[:N_CORES]
    mesh = Mesh(np.asarray(devices), ("core",))
    n_args = n_params + (n_outs if donate_outputs else 0)
    sharded = jax.jit(
        shard_map(_body, mesh=mesh,
                  in_specs=(PartitionSpec("core"),) * n_args,
                  out_specs=(PartitionSpec("core"),) * n_outs,
                  check_rep=False),
        donate_argnums=(tuple(range(n_params, n_params + n_outs))
                        if donate_outputs else ()),
        keep_unused=True,
    )

    def run(in_maps):
        concat_in = [
            np.concatenate([np.asarray(m[name]) for m in in_maps], axis=0)
            for name in in_names
        ]
        if donate_outputs:
            concat_in += [
                np.zeros((N_CORES * z.shape[0], *z.shape[1:]), z.dtype)
                for z in zero_outs
            ]
        out_arrs = sharded(*concat_in)
        return [
            {name: np.asarray(out_arrs[i]).reshape(
                N_CORES, *out_avals[i].shape)[c]
             for i, name in enumerate(out_names)}
            for c in range(N_CORES)
        ]

    return run


def get_runner():
    if 'runner' not in _CACHE:
        _CACHE['runner'] = _make_runner(_CACHE['prog'])
    return _CACHE['runner']


def _build_program(plan):
    import concourse.bacc as bacc
    import concourse.mybir as mybir
    from concourse.tile import TileContext
    from concourse.masks import make_identity

    DT = mybir.dt
    ALU = mybir.AluOpType
    slots = plan['slots']
    nc = bacc.Bacc("TRN2", target_bir_lowering=False, debug=False,
                   num_devices=N_CORES)
    rows_d, idx_d, wts_d, scl_d = [], [], [], []
    for k, sl in enumerate(slots):
        nidx = sl['nb'] * 128
        rows_d.append(nc.dram_tensor(f"rows{k}", [sl['ext'], sl['Wp'], 2],
                                     DT.int16, kind="ExternalInput").ap())
        idx_d.append(nc.dram_tensor(f"idx{k}", [2, 16, nidx // 16],
                                    DT.int16, kind="ExternalInput").ap())
        wts_d.append(nc.dram_tensor(f"wts{k}", [128, 6, sl['nb']],
                                    DT.float16, kind="ExternalInput").ap())
        scl_d.append(nc.dram_tensor(f"scl{k}", [128, 2],
                                    DT.float32, kind="ExternalInput").ap())
    accI_d = nc.dram_tensor("accI", [128, Z], DT.float16,
                            kind="ExternalOutput").ap()
    accQ_d = nc.dram_tensor("accQ", [128, Z], DT.float16,
                            kind="ExternalOutput").ap()

    with TileContext(nc) as tc:
        with tc.tile_pool(name="data", bufs=2) as dpool, \
             tc.tile_pool(name="small", bufs=2) as spool, \
             tc.tile_pool(name="gout", bufs=2) as gpool, \
             tc.tile_pool(name="unp", bufs=2) as upool, \
             tc.tile_pool(name="tmp", bufs=3) as tpool, \
             tc.tile_pool(name="accp", bufs=1) as apool, \
             tc.tile_pool(name="psum", bufs=2, space="PSUM") as ppool:
            ident = apool.tile([128, 128], DT.float32, tag="ident")
            make_identity(nc, ident[:])
            accI = apool.tile([128, NBLK, 128], DT.float32, tag="accI")
            accQ = apool.tile([128, NBLK, 128], DT.float32, tag="accQ")
            nc.vector.memset(accI[:], 0.0)
            nc.vector.memset(accQ[:], 0.0)

            for k, sl in enumerate(slots):
                ext, toff, blk0, Wp, nb = (sl['ext'], sl['toff'], sl['blk0'],
                                           sl['Wp'], sl['nb'])
                nidx = nb * 128
                c16 = nidx // 16
                data_t = dpool.tile([128, Wp, 2], DT.int16, tag="data")
                nc.vector.memset(data_t[:], 0)
                nc.sync.dma_start(out=data_t[0:ext], in_=rows_d[k][:])
                idx_t = spool.tile([128, 2 * c16], DT.int16, tag="idx")
                for r in range(8):
                    nc.sync.dma_start(out=idx_t[16 * r:16 * r + 16, 0:c16],
                                      in_=idx_d[k][0])
                    nc.sync.dma_start(out=idx_t[16 * r:16 * r + 16,
                                                c16:2 * c16],
                                      in_=idx_d[k][1])
                wts_t = spool.tile([128, 6, nb], DT.float16, tag="wts")
                nc.sync.dma_start(out=wts_t[:], in_=wts_d[k][:])
                scl_t = spool.tile([128, 2], DT.float32, tag="scl")
                nc.sync.dma_start(out=scl_t[:], in_=scl_d[k][:])

                for cst in range(0, nb, CB):
                    cb = min(CB, nb - cst)
                    cN = cb * 128
                    co16 = cst * 8          # column offset in idx table
                    ga = gpool.tile([128, CB * 128, 2], DT.int16, tag="ga")
                    gb = gpool.tile([128, CB * 128, 2], DT.int16, tag="gb")
                    nc.gpsimd.ap_gather(ga[:, 0:cN, :], data_t[:],
                                        idx_t[:, co16:co16 + cb * 8],
                                        channels=128, num_elems=Wp, d=2,
                                        num_idxs=cN)
                    nc.gpsimd.ap_gather(gb[:, 0:cN, :], data_t[:],
                                        idx_t[:, c16 + co16:
                                              c16 + co16 + cb * 8],
                                        channels=128, num_elems=Wp, d=2,
                                        num_idxs=cN)

                    # unpack a (both lanes) and b (lane 0 only)
                    low_a = upool.tile([128, CB * 128, 2], DT.int16, tag="la")
                    d_a = upool.tile([128, CB * 128, 2], DT.int16, tag="da")
                    I_a = upool.tile([128, CB * 128, 2], DT.float32, tag="Ia")
                    Q_a = upool.tile([128, CB * 128, 2], DT.float32, tag="Qa")
                    low_b = upool.tile([128, CB * 128], DT.int16, tag="lb")
                    d_b = upool.tile([128, CB * 128], DT.int16, tag="db")
                    I_b = upool.tile([128, CB * 128], DT.float32, tag="Ib")
                    Q_b = upool.tile([128, CB * 128], DT.float32, tag="Qb")
                    for (g_t, lo, dd, II, QQ) in (
                            (ga[:, 0:cN, :], low_a[:, 0:cN, :],
                             d_a[:, 0:cN, :], I_a[:, 0:cN, :],
                             Q_a[:, 0:cN, :]),
                            (gb[:, 0:cN, 0], low_b[:, 0:cN],
                             d_b[:, 0:cN], I_b[:, 0:cN], Q_b[:, 0:cN])):
                        nc.vector.tensor_scalar(out=lo, in0=g_t,
                                                scalar1=0x00FF, scalar2=None,
                                                op0=ALU.bitwise_and)
                        nc.vector.tensor_scalar(out=II, in0=lo,
                                                scalar1=128,
                                                scalar2=scl_t[:, 0:1],
                                                op0=ALU.subtract,
                                                op1=ALU.mult)
                        nc.vector.tensor_tensor(out=dd, in0=g_t, in1=lo,
                                                op=ALU.subtract)
                        nc.vector.tensor_scalar(out=QQ, in0=dd,
                                                scalar1=scl_t[:, 1:2],
                                                scalar2=None, op0=ALU.mult)

                    # 6 data streams -> transpose -> weighted accumulate
                    # (stream_view, accI table, accI sign, accQ table)
                    for (sv, tI, sgnI, tQ) in (
                            (I_a[:, 0:cN, 0], 0, True, 3),
                            (I_a[:, 0:cN, 1], 1, True, 4),
                            (I_b[:, 0:cN], 2, True, 5),
                            (Q_a[:, 0:cN, 0], 3, False, 0),
                            (Q_a[:, 0:cN, 1], 4, False, 1),
                            (Q_b[:, 0:cN], 5, False, 2)):
                        ps = ppool.tile([128, CB, 128], DT.float32,
                                        space="PSUM", tag="ps")
                        for bl in range(cb):
                            nc.tensor.transpose(
                                out=ps[:, bl, :],
                                in_=sv[:, bl * 128:(bl + 1) * 128],
                                identity=ident[:])
                        for (acc, tab, positive) in (
                                (accI, tI, sgnI), (accQ, tQ, True)):
                            tmp = tpool.tile([128, CB, 128], DT.float32,
                                             tag="tmp")
                            w_ap = wts_t[:, tab, cst:cst + cb] \
                                .broadcast_to([128, cb, ext])
                            nc.any.tensor_tensor(
                                out=tmp[:, 0:cb, 0:ext],
                                in0=ps[:, 0:cb, 0:ext], in1=w_ap,
                                op=ALU.mult)
                            asl = acc[:, blk0 + cst:blk0 + cst + cb,
                                      toff:toff + ext]
                            nc.any.tensor_tensor(
                                out=asl, in0=asl, in1=tmp[:, 0:cb, 0:ext],
                                op=ALU.add if positive else ALU.subtract)

            outI = apool.tile([128, NBLK, 128], DT.float16, tag="outI")
            outQ = apool.tile([128, NBLK, 128], DT.float16, tag="outQ")
            nc.vector.tensor_scalar(out=outI[:], in0=accI[:], scalar1=1.0,
                                    scalar2=None, op0=ALU.mult)
            nc.vector.tensor_scalar(out=outQ[:], in0=accQ[:], scalar1=1.0,
                                    scalar2=None, op0=ALU.mult)
            nc.sync.dma_start(out=accI_d[:], in_=outI[:])
            nc.sync.dma_start(out=accQ_d[:], in_=outQ[:])
    nc.compile()
    return nc


def get_program(tabs):
    if 'prog' not in _CACHE:
        plan = build_plan(tabs)
        _CACHE['plan'] = plan
        _CACHE['prog'] = _build_program(plan)
    return _CACHE['prog'], _CACHE['plan']


def kernel(idata, qdata, grid, tx_ori, ele_pos, time_zero,
           fs, c, fdemod, rxfnum):
    idata = _f32(idata); qdata = _f32(qdata)
    tabs = compute_tables(grid, tx_ori, ele_pos, time_zero,
                          fs, c, fdemod, rxfnum)
    nc, plan = get_program(tabs)
    in_maps = pack_inputs(idata, qdata, tabs, plan)
    results = get_runner()(in_maps)

    idas = np.zeros((T, Z), np.float32)
    qdas = np.zeros((T, Z), np.float32)
    for cidx in range(N_CORES):
        aI = results[cidx]["accI"].astype(np.float32) \
            .reshape(128, NBLK, 128)
        aQ = results[cidx]["accQ"].astype(np.float32) \
            .reshape(128, NBLK, 128)
        idas += aI.transpose(1, 0, 2).reshape(Z, T).T
        qdas += aQ.transpose(1, 0, 2).reshape(Z, T).T
    cI, cQ = corrections(idata, qdata, tabs)
    idas += cI
    qdas += cQ
    return (idas, qdas)
